# revision 29
# baseline (speedup 1.0000x reference)
"""GCN (3x GCNConv + BN + residual, mean-pool, MLP head) on 8 trn2 NeuronCores.

Sharding: nodes split contiguously across 8 cores (6250 each); each core owns
the edges whose TARGET lands in its shard (plus self-loops). Aggregation is
linear, so each layer aggregates prescaled source rows (x*dinv_src, stored
bf16) over incident edges, then applies the folded linear+BN epilogue. Layer
activations are AllGathered in two halves (bf16, two Shared dram tensors, so
the first AG overlaps the second half's compute); per-graph pooled sums are
AllReduced; the tiny MLP head is computed redundantly on every core.

All tables use one [half][core][local] row layout (halves split at local row
3136), so the three layers share a single gather-index/target-label table set
and int16 gather indices stay under 32768 per half.

Device kernel per (layer, target-block of 128 nodes):
  batched SWDGE dma_gathers (<=1024 rows each, spread round-robin over 4
    SWDGE queues) fetch the block's source rows from the two table halves;
  per 128-edge chunk: one-hot S[e,t] = (tl[e]==t) built on DVE in bf16;
    PE matmul agg[t,f] += S.T @ xrows (PSUM accumulate, bf16 operands)
  ACT copies agg out of PSUM scaled by dinv_tgt (folds GCN normalization);
  PE transposes agg -> [f,t]; h[t,o] = aggT.T @ W' (+bias via ones-row
  matmul into PSUM); ACT relu; DVE adds BN shift + residual in fp32.
"""
import math
import os
import sys

import numpy as np
import ml_dtypes

sys.path.insert(0, "/opt/trn_rl_repo")

N_NODES = 50000
N_EDGES = 800000
IN_DIM = 128
HID = 256
OUT_DIM = 1
N_GRAPHS = 512
BN_EPS = 1e-5
NCORES = 8
P = 128
SHARD = N_NODES // NCORES            # 6250
NBLK = (SHARD + P - 1) // P          # 49
PADN = NBLK * P                      # 6272 rows per core incl pad
XROWS = PADN * NCORES                # 50176 rows in all tables
LH = PADN // 2                       # 3136 local rows per half
HROWS = NCORES * LH                  # 25088 rows per table half


def _build_program(lsl, hsl):
    """lsl/hsl: per-block gather slot counts (mult of 16) for low/high half."""
    from concourse import bass, bacc, mybir, tile
    from concourse.masks import make_identity
    from concourse.library_config import mlp as mlp_lib

    f32 = mybir.dt.float32
    bf16 = mybir.dt.bfloat16
    i32 = mybir.dt.int32
    i16 = mybir.dt.int16
    AF = mybir.ActivationFunctionType
    OP = mybir.AluOpType

    lch = [(n + P - 1) // P for n in lsl]
    hch = [(n + P - 1) // P for n in hsl]
    cht = [l + h for l, h in zip(lch, hch)]
    MAXC = max(cht)
    MLW = max(n // 16 for n in lsl)      # idx cols per block, low
    MHW = max(n // 16 for n in hsl)

    nc = bacc.Bacc("TRN2", target_bir_lowering=False, debug=False,
                   num_devices=NCORES, num_swdge_queues=4)
    qrr = [0]

    def next_q():
        q = qrr[0]
        qrr[0] = (q + 1) % 4
        return q

    xta = nc.declare_dram_parameter("xta", [HROWS, IN_DIM], bf16, isOutput=False)
    xtb = nc.declare_dram_parameter("xtb", [HROWS, IN_DIM], bf16, isOutput=False)
    ilo = nc.declare_dram_parameter("ilo", [P, NBLK * MLW], i16, isOutput=False)
    ihi = nc.declare_dram_parameter("ihi", [P, NBLK * MHW], i16, isOutput=False)
    tlp = nc.declare_dram_parameter("tlp", [P, NBLK * MAXC], f32, isOutput=False)
    dcolp = nc.declare_dram_parameter("dcolp", [P, NBLK], f32, isOutput=False)
    bcolp = nc.declare_dram_parameter("bcolp", [P, NBLK], f32, isOutput=False)
    w1p = nc.declare_dram_parameter("w1p", [IN_DIM, HID], bf16, isOutput=False)
    w2p = nc.declare_dram_parameter("w2p", [HID, HID], bf16, isOutput=False)
    w3p = nc.declare_dram_parameter("w3p", [HID, HID], bf16, isOutput=False)
    browp = nc.declare_dram_parameter("browp", [3, HID], bf16, isOutput=False)
    tshp = nc.declare_dram_parameter("tshp", [P, 3 * HID], f32, isOutput=False)
    lw1 = nc.declare_dram_parameter("lw1", [HID, HID], f32, isOutput=False)
    lb1c = nc.declare_dram_parameter("lb1c", [P, 2], f32, isOutput=False)
    lw2 = nc.declare_dram_parameter("lw2", [P, 2], f32, isOutput=False)
    lb2c = nc.declare_dram_parameter("lb2c", [1, 1], f32, isOutput=False)
    icnt = nc.declare_dram_parameter("icnt", [P, N_GRAPHS], f32, isOutput=False)
    out = nc.declare_dram_parameter("out", [1, N_GRAPHS], f32, isOutput=True)

    with tile.TileContext(nc) as tc:
        with tc.tile_pool(name="const", bufs=1) as cpool, \
             tc.tile_pool(name="headp", bufs=1) as headp, \
             tc.tile_pool(name="rows", bufs=3) as rpool, \
             tc.tile_pool(name="smat", bufs=6) as spool, \
             tc.tile_pool(name="work", bufs=4) as wpool, \
             tc.tile_pool(name="resid", bufs=1) as residp, \
             tc.tile_pool(name="hrow", bufs=4) as hpool, \
             tc.tile_pool(name="psum", bufs=2, space="PSUM") as ppool, \
             tc.tile_pool(name="psump", bufs=1, space="PSUM") as ppoolp, \
             tc.tile_pool(name="dram", bufs=1, space="DRAM") as dpool:

            iota_i = cpool.tile([P, P], i32, tag="ioi")
            nc.gpsimd.iota(iota_i[:], pattern=[[1, P]], base=0, channel_multiplier=0)
            iota_bf = cpool.tile([P, P], bf16, tag="iob")
            nc.vector.tensor_copy(iota_bf[:], iota_i[:])
            iota5_i = cpool.tile([P, N_GRAPHS], i32, tag="io5i")
            nc.gpsimd.iota(iota5_i[:], pattern=[[1, N_GRAPHS]], base=0, channel_multiplier=0)
            iota5_f = cpool.tile([P, N_GRAPHS], f32, tag="io5f")
            nc.vector.tensor_copy(iota5_f[:], iota5_i[:])
            ident_f = cpool.tile([P, P], f32, tag="identf")
            make_identity(nc, ident_f[:])
            ident_bf = cpool.tile([P, P], bf16, tag="identb")
            nc.vector.tensor_copy(ident_bf[:], ident_f[:])
            ones1 = cpool.tile([1, P], bf16, tag="ones1")
            nc.vector.memset(ones1[:], 1.0)
            # iota (standard lib) done; dma_gather et al need the mlp library
            nc.gpsimd.load_library(mlp_lib)

            ilo_t = cpool.tile([P, NBLK * MLW], i16, tag="ilo")
            nc.sync.dma_start(out=ilo_t[:], in_=ilo[:, :])
            ihi_t = cpool.tile([P, NBLK * MHW], i16, tag="ihi")
            nc.sync.dma_start(out=ihi_t[:], in_=ihi[:, :])
            tl_t = cpool.tile([P, NBLK * MAXC], f32, tag="tlp")
            nc.sync.dma_start(out=tl_t[:], in_=tlp[:, :])
            dcol_t = cpool.tile([P, NBLK], f32, tag="dcol")
            nc.sync.dma_start(out=dcol_t[:], in_=dcolp[:, :])
            bcol_t = cpool.tile([P, NBLK], f32, tag="bcol")
            nc.sync.dma_start(out=bcol_t[:], in_=bcolp[:, :])

            w1_t = cpool.tile([IN_DIM, HID], bf16, tag="w1")
            nc.sync.dma_start(out=w1_t[:], in_=w1p[:, :])
            w2_t = [cpool.tile([P, HID], bf16, tag=f"w2_{k}", name=f"w2_{k}") for k in range(2)]
            w3_t = [cpool.tile([P, HID], bf16, tag=f"w3_{k}", name=f"w3_{k}") for k in range(2)]
            for k in range(2):
                nc.sync.dma_start(out=w2_t[k][:], in_=w2p[k * P:(k + 1) * P, :])
                nc.sync.dma_start(out=w3_t[k][:], in_=w3p[k * P:(k + 1) * P, :])
            brow_t = [cpool.tile([1, HID], bf16, tag=f"br{i}", name=f"br{i}") for i in range(3)]
            for i in range(3):
                nc.sync.dma_start(out=brow_t[i][:], in_=browp[i:i + 1, :])
            tsh_t = cpool.tile([P, 3 * HID], f32, tag="tsh")
            nc.sync.dma_start(out=tsh_t[:], in_=tshp[:, :])

            hloc1 = dpool.tile([PADN, HID], bf16, tag="hloc1")
            hloc2 = dpool.tile([PADN, HID], bf16, tag="hloc2")
            xn1a = dpool.tile([HROWS, HID], bf16, tag="xn1a", addr_space="Shared")
            xn1b = dpool.tile([HROWS, HID], bf16, tag="xn1b", addr_space="Shared")
            xn2a = dpool.tile([HROWS, HID], bf16, tag="xn2a", addr_space="Shared")
            xn2b = dpool.tile([HROWS, HID], bf16, tag="xn2b", addr_space="Shared")
            prdram = dpool.tile([HID, N_GRAPHS], f32, tag="prd")
            ardram = dpool.tile([HID, N_GRAPHS], f32, tag="ard")

            resid = [residp.tile([P, HID], f32, tag=f"r{b}", name=f"r{b}")
                     for b in range(NBLK)]

            pooled_ps = [ppoolp.tile([P, N_GRAPHS], f32, tag=f"pool{h}", name=f"pool{h}")
                         for h in range(2)]

            def layer(li, tablo, tabhi, fdim, wtiles, hloc):
                """One GCN layer. li: 0,1,2. tablo/tabhi: DRAM table halves.
                fdim: input width. wtiles: list of [128,HID] bf16 weight tiles.
                hloc: bf16 output row table or None (L3: pool inline)."""
                nf = fdim // P
                for b in range(NBLK):
                    lc, hc, ct = lch[b], hch[b], cht[b]
                    xr = rpool.tile([P, MAXC, fdim], bf16, tag="xr")
                    if b < 3:
                        # buffers start uninitialized; pad slots beyond
                        # num_idxs must hold finite bf16 (smat zeroes them)
                        nc.vector.memset(xr[:], 0.0)
                    # dma_gather caps at 1024 indices per instruction; the
                    # last segment's num_idxs is trimmed to a multiple of 16
                    # (slots past it stay stale and are zeroed by smat)
                    for s0 in range(0, lsl[b], 1024):
                        n = min(1024, lsl[b] - s0)
                        c0, c1 = s0 // P, (s0 + n + P - 1) // P
                        nc.gpsimd.dma_gather(
                            xr[:, c0:c1, :], tablo,
                            ilo_t[:, b * MLW + s0 // 16:b * MLW + (s0 + n) // 16],
                            n, n, fdim, queue_num=next_q())
                    for s0 in range(0, hsl[b], 1024):
                        n = min(1024, hsl[b] - s0)
                        c0, c1 = lc + s0 // P, lc + (s0 + n + P - 1) // P
                        nc.gpsimd.dma_gather(
                            xr[:, c0:c1, :], tabhi,
                            ihi_t[:, b * MHW + s0 // 16:b * MHW + (s0 + n) // 16],
                            n, n, fdim, queue_num=next_q())
                    agg_ps = ppool.tile([P, fdim], f32, tag="agg")
                    for j in range(ct):
                        smat = spool.tile([P, P], bf16, tag="smat")
                        nc.vector.tensor_scalar(
                            out=smat[:], in0=iota_bf[:],
                            scalar1=tl_t[:, b * MAXC + j:b * MAXC + j + 1],
                            scalar2=None, op0=OP.is_equal)
                        nc.tensor.matmul(
                            agg_ps[:], lhsT=smat[:], rhs=xr[:, j, :],
                            start=(j == 0), stop=(j == ct - 1))

                    aggs = wpool.tile([P, fdim], bf16, tag="aggs")
                    nc.scalar.mul(aggs[:], agg_ps[:], dcol_t[:, b:b + 1])

                    h_ps = ppool.tile([P, HID], f32, tag="h")
                    for k in range(nf):
                        tp_ps = ppool.tile([P, P], bf16, tag="tp", bufs=1)
                        nc.tensor.transpose(tp_ps[:], aggs[:, k * P:(k + 1) * P], ident_bf[:])
                        a2 = wpool.tile([P, P], bf16, tag=f"a2_{k}", name=f"a2_{k}")
                        nc.scalar.copy(a2[:], tp_ps[:])
                        nc.tensor.matmul(h_ps[:], lhsT=a2[:], rhs=wtiles[k][:],
                                         start=(k == 0), stop=False)
                    nc.tensor.matmul(h_ps[:], lhsT=ones1[:, :], rhs=brow_t[li][:],
                                     start=False, stop=True)

                    relu_sb = hpool.tile([P, HID], f32, tag="relu")
                    nc.scalar.activation(relu_sb[:], h_ps[:], AF.Relu)
                    tsl = tsh_t[:, li * HID:(li + 1) * HID]
                    if li == 0:
                        nc.vector.tensor_tensor(out=resid[b][:], in0=relu_sb[:],
                                                in1=tsl, op=OP.add)
                    else:
                        nc.vector.tensor_tensor(out=resid[b][:], in0=resid[b][:],
                                                in1=relu_sb[:], op=OP.add)
                        nc.vector.tensor_tensor(out=resid[b][:], in0=resid[b][:],
                                                in1=tsl, op=OP.add)

                    if hloc is not None:
                        tbl = hpool.tile([P, HID], bf16, tag="tbl")
                        nc.scalar.mul(tbl[:], resid[b][:], dcol_t[:, b:b + 1])
                        nc.sync.dma_start(out=hloc[b * P:(b + 1) * P, :], in_=tbl[:])
                    else:
                        mblk = spool.tile([P, N_GRAPHS], bf16, tag="mblk")
                        nc.vector.tensor_scalar(
                            out=mblk[:], in0=iota5_f[:],
                            scalar1=bcol_t[:, b:b + 1], scalar2=None,
                            op0=OP.is_equal)
                        residb = hpool.tile([P, HID], bf16, tag="residb")
                        nc.scalar.copy(residb[:], resid[b][:])
                        for h in range(2):
                            nc.tensor.matmul(
                                pooled_ps[h][:], lhsT=residb[:, h * P:(h + 1) * P],
                                rhs=mblk[:], start=(b == 0), stop=(b == NBLK - 1))

            def allgather2(hloc, xna, xnb):
                nc.gpsimd.collective_compute(
                    "AllGather", bass.mybir.AluOpType.bypass,
                    replica_groups=[list(range(NCORES))],
                    ins=[hloc[0:LH, :]], outs=[xna.opt()])
                nc.gpsimd.collective_compute(
                    "AllGather", bass.mybir.AluOpType.bypass,
                    replica_groups=[list(range(NCORES))],
                    ins=[hloc[LH:PADN, :]], outs=[xnb.opt()])

            layer(0, xta[:, :], xtb[:, :], IN_DIM, [w1_t], hloc1)
            allgather2(hloc1, xn1a, xn1b)
            layer(1, xn1a[:, :], xn1b[:, :], HID, w2_t, hloc2)
            allgather2(hloc2, xn2a, xn2b)
            layer(2, xn2a[:, :], xn2b[:, :], HID, w3_t, None)

            # pooled partial sums -> DRAM -> AllReduce
            icnt_t = cpool.tile([P, N_GRAPHS], f32, tag="icnt")
            nc.sync.dma_start(out=icnt_t[:], in_=icnt[:, :])
            for h in range(2):
                ps = headp.tile([P, N_GRAPHS], f32, tag=f"poolsb{h}", name=f"poolsb{h}")
                nc.vector.tensor_copy(ps[:], pooled_ps[h][:])
                nc.sync.dma_start(out=prdram[h * P:(h + 1) * P, :], in_=ps[:])
            nc.gpsimd.collective_compute(
                "AllReduce", bass.mybir.AluOpType.add,
                replica_groups=[list(range(NCORES))],
                ins=[prdram.opt()], outs=[ardram.opt()])

            # head: h1T[o,g] = relu(lw1.T @ (pooledT*icnt) + lb1); out = lw2.T @ h1T + lb2
            lw1_t = [cpool.tile([P, HID], f32, tag=f"lw1_{k}", name=f"lw1_{k}") for k in range(2)]
            lw2_t = cpool.tile([P, 2], f32, tag="lw2")
            lb1_t = cpool.tile([P, 2], f32, tag="lb1")
            lb2_t = cpool.tile([1, 1], f32, tag="lb2")
            for k in range(2):
                nc.sync.dma_start(out=lw1_t[k][:], in_=lw1[k * P:(k + 1) * P, :])
            nc.sync.dma_start(out=lw2_t[:], in_=lw2[:, :])
            nc.sync.dma_start(out=lb1_t[:], in_=lb1c[:, :])
            nc.sync.dma_start(out=lb2_t[:], in_=lb2c[:, :])

            par = []
            for k in range(2):
                pk = headp.tile([P, N_GRAPHS], f32, tag=f"par{k}", name=f"par{k}")
                nc.sync.dma_start(out=pk[:], in_=ardram[k * P:(k + 1) * P, :])
                pks = headp.tile([P, N_GRAPHS], f32, tag=f"pars{k}", name=f"pars{k}")
                nc.vector.tensor_tensor(out=pks[:], in0=pk[:], in1=icnt_t[:], op=OP.mult)
                par.append(pks)
            h1s = []
            for h in range(2):
                h1sb = headp.tile([P, N_GRAPHS], f32, tag=f"h1s{h}", name=f"h1s{h}")
                for half in range(2):
                    h1_ps = ppool.tile([P, HID], f32, tag="h")
                    gsl = slice(half * HID, (half + 1) * HID)
                    for k in range(2):
                        nc.tensor.matmul(h1_ps[:], lhsT=lw1_t[k][:, h * P:(h + 1) * P],
                                         rhs=par[k][:, gsl], start=(k == 0), stop=(k == 1))
                    nc.scalar.activation(h1sb[:, gsl], h1_ps[:], AF.Relu,
                                         bias=lb1_t[:, h:h + 1])
                h1s.append(h1sb)
            out_ps = ppool.tile([1, N_GRAPHS], f32, tag="outps", bufs=1)
            for h in range(2):
                nc.tensor.matmul(out_ps[:], lhsT=lw2_t[:, h:h + 1],
                                 rhs=h1s[h][:], start=(h == 0), stop=(h == 1))
            out_sb = headp.tile([1, N_GRAPHS], f32, tag="outs")
            nc.vector.tensor_scalar(out=out_sb[:], in0=out_ps[:],
                                    scalar1=lb2_t[0:1, 0:1], scalar2=None, op0=OP.add)
            nc.sync.dma_start(out=out[:, :], in_=out_sb[:])

    nc.compile()
    return nc


def _wrap16(lin):
    """int array (len multiple of 16) -> [128, n/16] wrapped + replicated."""
    w = lin.reshape(-1, 16).T  # [16, n/16]
    return np.tile(w, (8, 1)).astype(np.int16)


def _preprocess(x, edge_index, batch):
    """Per-core edge lists grouped by target block, split low/high source."""
    src = np.asarray(edge_index[0], dtype=np.int64)
    tgt = np.asarray(edge_index[1], dtype=np.int64)
    batch = np.asarray(batch, dtype=np.int64)

    deg = np.bincount(tgt, minlength=N_NODES).astype(np.float64) + 1.0
    dinv = 1.0 / np.sqrt(deg)

    allsrc = np.concatenate([src, np.arange(N_NODES, dtype=np.int64)])
    alltgt = np.concatenate([tgt, np.arange(N_NODES, dtype=np.int64)])

    # unified [half][core][local] table row for each source node
    scor = allsrc // SHARD
    sloc = allsrc % SHARD
    shalf = (sloc >= LH).astype(np.int64)
    remap = np.where(shalf == 0, scor * LH + sloc, scor * LH + (sloc - LH))

    # order edges by (target block, source half) in one sort
    coreid = alltgt // SHARD
    locid = alltgt - coreid * SHARD
    blkkey = (coreid * NBLK + locid // P) * 2 + shalf
    order = np.argsort(blkkey, kind="stable")
    allsrc, alltgt, remap, blkkey = (allsrc[order], alltgt[order],
                                     remap[order], blkkey[order])

    cnt2 = np.bincount(blkkey, minlength=NBLK * NCORES * 2)
    cnt2 = cnt2.reshape(NCORES, NBLK, 2)
    # per-block gather slot counts (mult of 16), uniform across cores (SPMD)
    lsl = [int(-(-cnt2[:, b, 0].max() // 16) * 16) for b in range(NBLK)]
    hsl = [int(-(-cnt2[:, b, 1].max() // 16) * 16) for b in range(NBLK)]
    lch = [(n + P - 1) // P for n in lsl]
    hch = [(n + P - 1) // P for n in hsl]
    cht = [l + h for l, h in zip(lch, hch)]
    MAXC = max(cht)
    MLW = max(n // 16 for n in lsl)
    MHW = max(n // 16 for n in hsl)

    start2 = np.zeros(NBLK * NCORES * 2 + 1, dtype=np.int64)
    np.cumsum(cnt2.reshape(-1), out=start2[1:])

    per_core = []
    for c in range(NCORES):
        ilo = np.zeros((P, NBLK * MLW), dtype=np.int16)
        ihi = np.zeros((P, NBLK * MHW), dtype=np.int16)
        tl = np.full((P, NBLK * MAXC), -1.0, dtype=np.float32)
        for b in range(NBLK):
            g = (c * NBLK + b) * 2
            tlb = np.full(cht[b] * P, -1.0, dtype=np.float32)
            halves = ((lsl[b], MLW, ilo, 0),
                      (hsl[b], MHW, ihi, lch[b] * P))
            for half, (nsl, mw, itab, toff) in enumerate(halves):
                lo, hi = start2[g + half], start2[g + half + 1]
                n = hi - lo
                s2 = np.pad(remap[lo:hi], (0, nsl - n)).astype(np.int64)
                itab[:, b * mw:b * mw + nsl // 16] = _wrap16(s2)
                tlb[toff:toff + n] = (alltgt[lo:hi] - (c * SHARD + b * P)).astype(np.float32)
            tl[:, b * MAXC:b * MAXC + cht[b]] = tlb.reshape(cht[b], P).T
        # batch + dinv columns for this core's blocks (pad rows -> -1 / 1.0)
        nloc = np.arange(c * SHARD, (c + 1) * SHARD)
        bvals = np.pad(batch[nloc].astype(np.float32), (0, PADN - SHARD),
                       constant_values=-1.0)
        dvals = np.pad(dinv[nloc].astype(np.float32), (0, PADN - SHARD),
                       constant_values=1.0)
        per_core.append(dict(
            ilo=ilo, ihi=ihi,
            tlp=tl,
            bcolp=np.ascontiguousarray(bvals.reshape(NBLK, P).T),
            dcolp=np.ascontiguousarray(dvals.reshape(NBLK, P).T.astype(np.float32)),
        ))
    return per_core, lsl, hsl, dinv


def kernel(**inputs):
    from concourse.bass_utils import run_bass_kernel_spmd

    x = np.asarray(inputs["x"], dtype=np.float32)
    edge_index = np.asarray(inputs["edge_index"])
    batch = np.asarray(inputs["batch"])

    per_core, lsl, hsl, dinv = _preprocess(x, edge_index, batch)

    def g(k):
        return np.asarray(inputs[k], dtype=np.float32)

    params = {}
    # x*dinv in the [half][core][local] table layout (pad rows zero)
    xt = (x * dinv[:, None].astype(np.float32)).astype(ml_dtypes.bfloat16)
    xta = np.zeros((HROWS, IN_DIM), ml_dtypes.bfloat16)
    xtb = np.zeros((HROWS, IN_DIM), ml_dtypes.bfloat16)
    for c in range(NCORES):
        lo = c * SHARD
        xta[c * LH:c * LH + LH] = xt[lo:lo + LH]
        nhi = SHARD - LH             # 3114 real rows in the high half
        xtb[c * LH:c * LH + nhi] = xt[lo + LH:lo + SHARD]
    params["xta"], params["xtb"] = xta, xtb

    Ws = [g("W1"), g("W2"), g("W3")]
    bs = [g("b1"), g("b2"), g("b3")]
    brow = np.zeros((3, HID), np.float32)
    tshp = np.zeros((P, 3 * HID), np.float32)
    wp = []
    for i in range(3):
        gam, be, m, v = g(f"g{i+1}"), g(f"be{i+1}"), g(f"m{i+1}"), g(f"v{i+1}")
        s = gam / np.sqrt(v + BN_EPS)
        assert (s > 0).all(), "BN scale must be positive for relu folding"
        wp.append((Ws[i] * s[None, :]).astype(ml_dtypes.bfloat16))
        brow[i] = bs[i] * s
        tshp[:, i * HID:(i + 1) * HID] = (be - m * s)[None, :]
    params["w1p"], params["w2p"], params["w3p"] = wp
    params["browp"] = brow.astype(ml_dtypes.bfloat16)
    params["tshp"] = tshp
    params["lw1"] = g("lw1")
    lb1 = g("lb1")
    lb1c = np.zeros((P, 2), np.float32)
    lb1c[:, 0] = lb1[:P]
    lb1c[:, 1] = lb1[P:]
    params["lb1c"] = lb1c
    lw2v = g("lw2").reshape(HID)
    params["lw2"] = np.stack([lw2v[:P], lw2v[P:]], axis=1).copy()
    params["lb2c"] = g("lb2").reshape(1, 1).astype(np.float32)
    cnt = np.bincount(np.asarray(batch, dtype=np.int64), minlength=N_GRAPHS)
    icnt = (1.0 / np.maximum(cnt, 1)).astype(np.float32)
    params["icnt"] = np.tile(icnt[None, :], (P, 1))

    nc = _build_program(lsl, hsl)

    in_maps = []
    for c in range(NCORES):
        m = dict(params)
        m.update(per_core[c])
        in_maps.append(m)

    res = run_bass_kernel_spmd(nc, in_maps, list(range(NCORES)),
                               trace=bool(os.environ.get("GNN_TRACE")))
    if os.environ.get("GNN_TRACE"):
        print("HW exec time:", res.exec_time_ns, "ns")
    global _last_results, _last_res
    _last_results = res.results
    _last_res = res
    o = res.results[0]["out"]
    return np.asarray(o, dtype=np.float32).reshape(N_GRAPHS, OUT_DIM)
